# revision 1
# baseline (speedup 1.0000x reference)
"""Trainium2 Bass kernel for nn_BioGNN (3-layer GAT + mean-pool + linear head).

v2: bf16 edge-phase data path.
  - hrow rows are bf16 [h(256) | e_src(4) | e_dst(4) | pad] with ROWP=384
    elements (768B, dma_gather needs elem bytes %256==0).
  - denom fused into the numer scatter matmul (msg gets 4 ex columns).
  - per-tile VE ops batched into per-block ops via stride-0 broadcast APs.
  - gathers spread across SWDGE queues with a larger descriptor carveout.
"""
import sys

for _p in ("/opt/trn_rl_repo", "/root/.axon_site/_ro/trn_rl_repo"):
    if _p not in sys.path:
        sys.path.insert(0, _p)

import numpy as np

import concourse.bass as bass
import concourse.tile as tile
from concourse import bacc, mybir
from concourse.bass import broadcast_tensor_aps
from concourse.bass_utils import run_bass_kernel_spmd
from concourse.library_config import mlp as mlp_lib

P = 128
NCORES = 8
FDT = mybir.dt.float32
BDT = mybir.dt.bfloat16
I16 = mybir.dt.int16
AF = mybir.ActivationFunctionType
ALU = mybir.AluOpType
NPB = mybir.dt.np(BDT)

CFG = dict(N=50000, G=64, IN=128, HID=64, H=4, OUT=10)
ABLATE = set()
ROWP = 384  # bf16 row: 264 used, padded to 768B (256B-multiple)
NGQ = 4     # SWDGE queues to spread gathers over


def build_program(TBS, cfg=CFG):
    N, G, IN, HID, H, OUTF = (cfg["N"], cfg["G"], cfg["IN"], cfg["HID"],
                              cfg["H"], cfg["OUT"])
    F = H * HID
    ROW = F + 8
    NSH = N // NCORES
    NB = (NSH + P - 1) // P
    NSHP = NB * P
    HALF = NCORES * NSHP // 2
    assert len(TBS) == NB and all(len(t) == 2 for t in TBS)
    TBSUM = [lo + hi for lo, hi in TBS]
    TT = sum(TBSUM)
    KT = F // P
    NCALLS = sum((tg + 7) // 8 for lo_hi in TBS for tg in lo_hi if tg)

    nc = bacc.Bacc("TRN2", target_bir_lowering=False, debug=False,
                   num_devices=NCORES, dynamic_dma_scratch_size=65536,
                   num_swdge_queues=NGQ)

    # ---- I/O ----
    xT = nc.dram_tensor("xT", [IN, NSHP], BDT, kind="ExternalInput")
    idx16 = nc.dram_tensor("idx16", [P, 8 * TT], I16, kind="ExternalInput")
    dstrow = nc.dram_tensor("dstrow", [1, TT * P], BDT, kind="ExternalInput")
    iotap_in = nc.dram_tensor("iotap", [P, 1], FDT, kind="ExternalInput")
    dstloc = nc.dram_tensor("dstloc", [P, TT], BDT, kind="ExternalInput")
    batchloc = nc.dram_tensor("batchloc", [P, NB], FDT, kind="ExternalInput")
    iota_in = nc.dram_tensor("iota", [P, P], BDT, kind="ExternalInput")
    ident_in = nc.dram_tensor("ident", [P, P], BDT, kind="ExternalInput")
    wts = [nc.dram_tensor(f"wt{l}", [IN if l == 1 else F, ROW], BDT,
                          kind="ExternalInput") for l in (1, 2, 3)]
    breps = [nc.dram_tensor(f"brep{l}", [P, F], BDT, kind="ExternalInput")
             for l in (1, 2, 3)]
    wlt = nc.dram_tensor("wlt", [F, OUTF], BDT, kind="ExternalInput")
    blrep = nc.dram_tensor("blrep", [G, OUTF], FDT, kind="ExternalInput")
    invcnt = nc.dram_tensor("invcnt", [G, 1], FDT, kind="ExternalInput")
    gcnt = nc.dram_tensor("gcnt", [1, NCALLS], mybir.dt.int32,
                          kind="ExternalInput")
    out_ext = nc.dram_tensor("out", [G, OUTF], FDT, kind="ExternalOutput")

    # ---- internal DRAM ----
    hrow_own = nc.dram_tensor("hrow_own", [NSHP, ROWP], BDT)
    hrow_full = nc.dram_tensor("hrow_full", [NCORES * NSHP, ROWP], BDT,
                               addr_space="Shared")
    hT_own = [nc.dram_tensor(f"hT_own{l}", [F, NSHP], BDT) for l in (1, 2)]
    pool_own = nc.dram_tensor("pool_own", [G, F], FDT)
    pool_full = nc.dram_tensor("pool_full", [G, F], FDT, addr_space="Shared")

    with tile.TileContext(nc) as tc:
        with (
            tc.tile_pool(name="const", bufs=1) as cpool,
            tc.tile_pool(name="wpool", bufs=2) as wpool,
            tc.tile_pool(name="sb", bufs=4) as pool,
            tc.tile_pool(name="gpool", bufs=2) as gpool,
            tc.tile_pool(name="ps", bufs=2, space="PSUM") as pspool,
            tc.tile_pool(name="pspool1", bufs=1, space="PSUM") as pspool1,
        ):
            # resident constants
            iota_sb = cpool.tile([P, P], BDT)
            nc.sync.dma_start(iota_sb[:], iota_in[:])
            ident_sb = cpool.tile([P, P], BDT)
            nc.sync.dma_start(ident_sb[:], ident_in[:])
            idx16_sb = cpool.tile([P, 8 * TT], I16)
            nc.sync.dma_start(idx16_sb[:], idx16[:])
            iotap_sb = cpool.tile([P, 1], FDT)
            nc.sync.dma_start(iotap_sb[:], iotap_in[:])
            dstloc_sb = cpool.tile([P, TT], BDT)
            nc.sync.dma_start(dstloc_sb[:], dstloc[:])
            batchloc_sb = cpool.tile([P, NB], FDT)
            nc.sync.dma_start(batchloc_sb[:], batchloc[:])
            gcnt_sb = cpool.tile([1, NCALLS], mybir.dt.int32)
            nc.sync.dma_start(gcnt_sb[:], gcnt[:])

            nc.gpsimd.load_library(mlp_lib)
            zcol_sb = cpool.tile([P, ROWP - ROW], BDT)
            nc.vector.memset(zcol_sb[:], 0.0)
            for b in range(NB):
                nc.sync.dma_start(hrow_own[b * P:(b + 1) * P, ROW:], zcol_sb[:])
            TBMAX = max(TBSUM)
            for _gi in range(2):
                g0 = gpool.tile([P, TBMAX * ROWP], BDT, tag="gath")
                nc.vector.memset(g0[:], 0.0)
            tc.strict_bb_all_engine_barrier()

            pool_ps = pspool1.tile([G, F], mybir.dt.float32, tag="pool")

            for layer in (1, 2, 3):
                kt = 1 if layer == 1 else KT
                wt_sb = []
                for k in range(kt):
                    w = wpool.tile([P, ROW], BDT, tag=f"wt{k}")
                    nc.sync.dma_start(w[:], wts[layer - 1][k * P:(k + 1) * P, :])
                    wt_sb.append(w)
                brep_sb = wpool.tile([P, F], BDT, tag="brep")
                nc.sync.dma_start(brep_sb[:], breps[layer - 1][:])

                # ---- phase A: dense + write hrow_own ----
                for b in range(NB):
                    hlin_ps = pspool.tile([P, ROW], mybir.dt.float32, tag="mm")
                    for k in range(kt):
                        lt = pool.tile([P, P], BDT, tag="lhsT")
                        if layer == 1:
                            nc.scalar.dma_start(lt[:], xT[:, b * P:(b + 1) * P])
                        else:
                            nc.scalar.dma_start(
                                lt[:],
                                hT_own[layer - 2][k * P:(k + 1) * P,
                                                  b * P:(b + 1) * P])
                        nc.tensor.matmul(hlin_ps[:], lhsT=lt[:], rhs=wt_sb[k][:],
                                         start=(k == 0), stop=(k == kt - 1))
                    hrow_sb = pool.tile([P, ROW], BDT, tag="hrow")
                    nc.vector.tensor_copy(hrow_sb[:], hlin_ps[:])
                    nc.sync.dma_start(hrow_own[b * P:(b + 1) * P, :ROW], hrow_sb[:])

                # ---- phase B: AllGather rows ----
                if "noag" not in ABLATE:
                    nc.gpsimd.collective_compute(
                        "AllGather", ALU.bypass,
                        ins=[hrow_own[:]], outs=[hrow_full[:]],
                        replica_groups=[list(range(NCORES))],
                    )

                # ---- phase C: edge phase ----
                t0 = 0
                qn = 0
                ci = 0
                for b in range(NB):
                    Tb = TBSUM[b]
                    gath = gpool.tile([P, Tb * ROWP], BDT, tag="gath")
                    g3 = gath[:].rearrange("p (t r) -> p t r", r=ROWP)
                    goff = 0
                    for half in (0, 1):
                        Tg = TBS[b][half]
                        if Tg == 0:
                            continue
                        if "gather" in ABLATE:
                            if half == 0:
                                nc.gpsimd.memset(gath[:], 0.0)
                            goff += Tg
                            continue
                        for _rep in range(2 if "gath2" in ABLATE else 1):
                            done = 0
                            while done < Tg:
                                ck = min(8, Tg - done)
                                o = goff + done
                                nc.gpsimd.dma_gather(
                                    out_ap=gath[:, o * ROWP:(o + ck) * ROWP]
                                        .rearrange("p (t e) -> p t e", e=ROWP),
                                    in_ap=hrow_full[half * HALF:(half + 1) * HALF, :],
                                    idxs_ap=idx16_sb[:, 8 * (t0 + o):8 * (t0 + o + ck)],
                                    num_idxs=ck * P,
                                    num_idxs_reg=ck * P,
                                    elem_size=ROWP,
                                    queue_num=qn,
                                    single_packet=True,
                                )
                                qn = (qn + 1) % NGQ
                                done += ck
                                ci += 1
                        goff += Tg

                    # e_dst path: drep bcast -> ptall one-hot^T -> per-tile MMs
                    dstrow_sb = pool.tile([1, Tb * P], BDT, tag="dstrow")
                    nc.scalar.dma_start(dstrow_sb[:],
                                      dstrow[0:1, t0 * P:(t0 + Tb) * P])
                    drep = gpool.tile([P, Tb * P], BDT, tag="drep")
                    for _rep in range(2 if "drep2" in ABLATE else 1):
                        nc.gpsimd.partition_broadcast(drep[:], dstrow_sb[:])
                    ptall = gpool.tile([P, Tb * P], BDT, tag="ptall")
                    for _rep in range(2 if "ptall2" in ABLATE else 1):
                        nc.vector.tensor_scalar(
                            out=ptall[:], in0=drep[:], scalar1=iotap_sb[:, 0:1],
                            scalar2=None, op0=ALU.is_equal)
                    edb = pool.tile([P, 4], BDT, tag="edb")
                    nc.scalar.dma_start(edb[:],
                                      hrow_own[b * P:(b + 1) * P, F + 4:F + 8])
                    edst_ps = pspool.tile([P, 4 * Tb], mybir.dt.float32,
                                          tag="mm")
                    for _rep in range(2 if "edstmm2" in ABLATE else 1):
                        for t in range(Tb):
                            nc.tensor.matmul(edst_ps[:, 4 * t:4 * t + 4],
                                             lhsT=ptall[:, t * P:(t + 1) * P],
                                             rhs=edb[:], start=True, stop=True)

                    # pmat (one-hot scatter matrices) for all tiles: 1 VE op
                    pmat = gpool.tile([P, Tb * P], BDT, tag="pmat")
                    pm3 = pmat[:].rearrange("p (t c) -> p t c", c=P)
                    io3 = iota_sb[:].rearrange("p (t c) -> p t c", t=1)
                    dl3 = dstloc_sb[:, t0:t0 + Tb].rearrange("p (t c) -> p t c",
                                                             c=1)
                    a, bb = broadcast_tensor_aps(io3, dl3)
                    for _rep in range(2 if "pmat2" in ABLATE else 1):
                        nc.vector.tensor_tensor(out=pm3, in0=a, in1=bb,
                                                op=ALU.is_equal)

                    # logits = e_src(gathered) + e_dst(expanded)
                    lg = pool.tile([P, 4 * Tb], BDT, tag="lg")
                    nc.vector.tensor_tensor(
                        out=lg[:].rearrange("p (t f) -> p t f", f=4),
                        in0=g3[:, :, F:F + 4],
                        in1=edst_ps[:].rearrange("p (t f) -> p t f", f=4),
                        op=ALU.add)
                    lr = pool.tile([P, 4 * Tb], BDT, tag="lr")
                    nc.vector.scalar_tensor_tensor(
                        out=lr[:], in0=lg[:], scalar=0.2, in1=lg[:],
                        op0=ALU.mult, op1=ALU.max)
                    nc.vector.tensor_scalar_min(lr[:], lr[:], 60.0)
                    ex = pool.tile([P, 4 * Tb], BDT, tag="ex")
                    nc.scalar.activation(ex[:], lr[:], AF.Exp)
                    ex3 = ex[:].rearrange("p (t f) -> p t f", f=4)

                    # msg = [h_src * ex_h | ex] : 4 head muls + 1 copy
                    msg = gpool.tile([P, Tb * (F + 4)], BDT, tag="msg")
                    m3 = msg[:].rearrange("p (t c) -> p t c", c=F + 4)
                    for _rep in range(2 if "msg2" in ABLATE else 1):
                        for h in range(H):
                            a, bb = broadcast_tensor_aps(
                                g3[:, :, h * HID:(h + 1) * HID],
                                ex3[:, :, h:h + 1])
                            nc.vector.tensor_tensor(
                                out=m3[:, :, h * HID:(h + 1) * HID],
                                in0=a, in1=bb, op=ALU.mult)
                    nc.vector.tensor_copy(m3[:, :, F:F + 4], ex3)

                    # scatter: numer+denom in one accumulating matmul chain
                    nd_ps = pspool.tile([P, F + 4], mybir.dt.float32, tag="mm")
                    for _rep in range(2 if "scat2" in ABLATE else 1):
                        for t in range(Tb):
                            nc.tensor.matmul(nd_ps[:],
                                             lhsT=pmat[:, t * P:(t + 1) * P],
                                             rhs=msg[:, t * (F + 4):(t + 1) * (F + 4)],
                                             start=(t == 0), stop=(t == Tb - 1))

                    # self-loop diagonal path: numer += exs*h_own,
                    # denom += exs, straight from hrow_own (no gather/scatter)
                    hself = pool.tile([P, ROW], BDT, tag="hself")
                    nc.scalar.dma_start(hself[:],
                                        hrow_own[b * P:(b + 1) * P, :ROW])
                    lgs = pool.tile([P, 4], BDT, tag="lgs")
                    nc.vector.tensor_tensor(out=lgs[:], in0=hself[:, F:F + 4],
                                            in1=hself[:, F + 4:F + 8],
                                            op=ALU.add)
                    lrs = pool.tile([P, 4], BDT, tag="lrs")
                    nc.vector.scalar_tensor_tensor(
                        out=lrs[:], in0=lgs[:], scalar=0.2, in1=lgs[:],
                        op0=ALU.mult, op1=ALU.max)
                    exs = pool.tile([P, 4], FDT, tag="exs")
                    nc.scalar.activation(exs[:], lrs[:], AF.Exp)
                    sm = pool.tile([P, F + 4], BDT, tag="sm")
                    for h in range(H):
                        nc.vector.tensor_scalar_mul(
                            sm[:, h * HID:(h + 1) * HID],
                            hself[:, h * HID:(h + 1) * HID],
                            exs[:, h:h + 1])
                    nc.vector.tensor_copy(sm[:, F:F + 4], exs[:])
                    nc.vector.tensor_tensor(out=nd_ps[:], in0=nd_ps[:],
                                            in1=sm[:], op=ALU.add)

                    # epilogue: y = numer/denom + b
                    dsum = pool.tile([P, 4], FDT, tag="dsum")
                    nc.vector.tensor_scalar_max(dsum[:], nd_ps[:, F:F + 4], 1e-12)
                    rec = pool.tile([P, 4], FDT, tag="rec")
                    nc.vector.reciprocal(rec[:], dsum[:])
                    y = pool.tile([P, F], BDT, tag="y")
                    for h in range(H):
                        nc.vector.tensor_scalar_mul(
                            y[:, h * HID:(h + 1) * HID],
                            nd_ps[:, h * HID:(h + 1) * HID],
                            rec[:, h:h + 1])
                    nc.vector.tensor_tensor(out=y[:], in0=y[:], in1=brep_sb[:],
                                            op=ALU.add)
                    if layer < 3:
                        # ELU: relu(y) + exp(min(y,0)) - 1
                        mn = pool.tile([P, F], BDT, tag="mn")
                        nc.vector.tensor_scalar_min(mn[:], y[:], 0.0)
                        eu = pool.tile([P, F], BDT, tag="eu")
                        nc.scalar.activation(eu[:], mn[:], AF.Exp)
                        rl = pool.tile([P, F], BDT, tag="rl")
                        nc.scalar.activation(rl[:], y[:], AF.Relu)
                        hv = pool.tile([P, F], BDT, tag="hv")
                        nc.vector.scalar_tensor_tensor(
                            out=hv[:], in0=eu[:], scalar=-1.0, in1=rl[:],
                            op0=ALU.add, op1=ALU.add)
                        for k in range(KT):
                            tp = pspool.tile([P, P], BDT, tag="tp")
                            nc.tensor.transpose(tp[:], hv[:, k * P:(k + 1) * P],
                                                ident_sb[:])
                            tps = pool.tile([P, P], BDT, tag="tps")
                            nc.vector.tensor_copy(tps[:], tp[:])
                            nc.sync.dma_start(
                                hT_own[layer - 1][k * P:(k + 1) * P,
                                                  b * P:(b + 1) * P], tps[:])
                    else:
                        bmat = pool.tile([P, G], BDT, tag="bmat")
                        nc.vector.tensor_scalar(
                            out=bmat[:], in0=iota_sb[:, :G],
                            scalar1=batchloc_sb[:, b:b + 1],
                            scalar2=None, op0=ALU.is_equal)
                        ymm = pool.tile([P, F], BDT, tag="ymm")
                        nc.vector.tensor_copy(ymm[:], y[:])
                        nc.tensor.matmul(pool_ps[:], lhsT=bmat[:], rhs=ymm[:],
                                         start=(b == 0), stop=(b == NB - 1))
                    t0 += Tb

            # ---- final: pool -> AllReduce -> mean -> linear ----
            pool_sb = pool.tile([G, F], FDT, tag="poolsb")
            nc.vector.tensor_copy(pool_sb[:], pool_ps[:])
            nc.sync.dma_start(pool_own[:], pool_sb[:])
            nc.gpsimd.collective_compute(
                "AllReduce", ALU.add,
                ins=[pool_own[:]], outs=[pool_full[:]],
                replica_groups=[list(range(NCORES))],
            )
            invcnt_sb = cpool.tile([G, 1], FDT)
            nc.sync.dma_start(invcnt_sb[:], invcnt[:])
            wlt_sb = []
            for k in range(KT):
                w = cpool.tile([P, OUTF], BDT)
                nc.sync.dma_start(w[:], wlt[k * P:(k + 1) * P, :])
                wlt_sb.append(w)
            blrep_sb = cpool.tile([G, OUTF], FDT)
            nc.sync.dma_start(blrep_sb[:], blrep[:])

            pooled = pool.tile([G, F], FDT, tag="pooled")
            nc.sync.dma_start(pooled[:], pool_full[:])
            mean = pool.tile([G, F], BDT, tag="mean")
            nc.vector.tensor_scalar_mul(mean[:], pooled[:], invcnt_sb[:])
            fin_ps = pspool.tile([G, OUTF], mybir.dt.float32, tag="mm")
            for k in range(KT):
                ptp = pspool.tile([P, G], BDT, tag="tp")
                nc.tensor.transpose(ptp[:], mean[:, k * P:(k + 1) * P],
                                    ident_sb[:G, :G])
                ptps = pool.tile([P, G], BDT, tag="ptps")
                nc.vector.tensor_copy(ptps[:], ptp[:])
                nc.tensor.matmul(fin_ps[:], lhsT=ptps[:], rhs=wlt_sb[k][:],
                                 start=(k == 0), stop=(k == KT - 1))
            outv = pool.tile([G, OUTF], FDT, tag="outv")
            nc.vector.tensor_tensor(out=outv[:], in0=fin_ps[:], in1=blrep_sb[:],
                                    op=ALU.add)
            nc.sync.dma_start(out_ext[:], outv[:])

    nc.compile()
    return nc


def preprocess(x, edge_index, batch, params, cfg=CFG):
    """Host-side index preprocessing + param packing -> (TBS, in_maps)."""
    N, G, IN, HID, H, OUTF = (cfg["N"], cfg["G"], cfg["IN"], cfg["HID"],
                              cfg["H"], cfg["OUT"])
    F = H * HID
    NSH = N // NCORES
    NB = (NSH + P - 1) // P
    NSHP = NB * P

    HALF = NCORES * NSHP // 2
    src = np.asarray(edge_index[0]).astype(np.int64)
    dst = np.asarray(edge_index[1]).astype(np.int64)
    batch = np.asarray(batch).astype(np.int64)

    def remap(nodes):
        return (nodes // NSH) * NSHP + nodes % NSH

    core_of = dst // NSH
    tiles_lo = np.zeros((NCORES, NB), np.int64)
    tiles_hi = np.zeros((NCORES, NB), np.int64)
    per_core = []
    for c in range(NCORES):
        m = core_of == c
        s_c, d_c = remap(src[m]), dst[m] - c * NSH
        half_c = (s_c >= HALF).astype(np.int64)
        blk = d_c // P
        order = np.lexsort((d_c, half_c, blk))
        s_c, d_c, half_c, blk = s_c[order], d_c[order], half_c[order], blk[order]
        cnt_lo = np.bincount(blk[half_c == 0], minlength=NB)
        cnt_hi = np.bincount(blk[half_c == 1], minlength=NB)
        tiles_lo[c] = (cnt_lo + P - 1) // P
        tiles_hi[c] = (cnt_hi + P - 1) // P
        per_core.append((s_c, d_c, half_c, blk, cnt_lo, cnt_hi))
    TBS = [(int(max(tiles_lo[:, b].max(), 1)), int(tiles_hi[:, b].max()))
           for b in range(NB)]
    TBSUM = [lo + hi for lo, hi in TBS]
    TT = sum(TBSUM)
    tb0 = np.cumsum([0] + TBSUM[:-1])
    tbhi0 = [tb0[b] + TBS[b][0] for b in range(NB)]

    W = {k: np.asarray(v, np.float64) for k, v in params.items()}
    wt_aug = {}
    for l in (1, 2, 3):
        Wl = W[f"W{l}"]
        asrc, adst = W[f"a_src{l}"], W[f"a_dst{l}"]
        Ablk_s = np.zeros((F, H))
        Ablk_d = np.zeros((F, H))
        for h in range(H):
            Ablk_s[h * HID:(h + 1) * HID, h] = asrc[h]
            Ablk_d[h * HID:(h + 1) * HID, h] = adst[h]
        wt_aug[l] = np.concatenate(
            [Wl.T, Wl.T @ Ablk_s, Wl.T @ Ablk_d], axis=1).astype(NPB)

    counts = np.bincount(batch, minlength=G).astype(np.float64)
    invcnt = (1.0 / np.maximum(counts, 1.0)).astype(np.float32)[:, None]
    iota = np.tile(np.arange(P, dtype=np.float32), (P, 1)).astype(NPB)
    ident = np.eye(P, dtype=np.float32).astype(NPB)

    in_maps = []
    xarr = np.asarray(x)
    for c in range(NCORES):
        s_c, d_c, half_c, blk, cnt_lo, cnt_hi = per_core[c]
        grp_key = blk * 2 + half_c
        grp_cnt = np.bincount(grp_key, minlength=2 * NB)
        grp_start = np.concatenate([[0], np.cumsum(grp_cnt)[:-1]])
        pos_in_grp = np.arange(len(d_c)) - grp_start[grp_key]
        grp_t0 = np.where(half_c == 0, tb0[blk], np.asarray(tbhi0)[blk])
        t_idx = (grp_t0 + pos_in_grp // P).astype(np.int64)
        p_idx = (pos_in_grp % P).astype(np.int64)

        dstloc = np.full((P, TT), -1.0, np.float32)
        dstloc[p_idx, t_idx] = (d_c - blk * P).astype(np.float32)
        dstrow = np.ascontiguousarray(dstloc.T).reshape(1, TT * P).astype(NPB)

        idxflat = np.zeros(TT * P, np.int16)
        idxflat[t_idx * P + p_idx] = (s_c - half_c * HALF).astype(np.int16)
        gcnts = []
        for b in range(NB):
            for half in (0, 1):
                Tg = TBS[b][half]
                if Tg == 0:
                    continue
                gt0 = tb0[b] if half == 0 else tbhi0[b]
                done = 0
                while done < Tg:
                    ck = min(8, Tg - done)
                    s0 = (gt0 + done) * P
                    sl = idxflat[s0:s0 + ck * P]
                    nvalid = int((sl >= 0).sum())
                    if nvalid == 0:
                        idxflat[s0] = 0
                        nvalid = 1
                    else:
                        last = np.nonzero(sl >= 0)[0][-1]
                        assert (sl[:last + 1] >= 0).all(), "pads not trailing"
                    gcnts.append(nvalid)
                    done += ck
        gcnt_arr = np.asarray(gcnts, np.int32)[None, :]
        idx16 = np.ascontiguousarray(
            np.tile(idxflat.reshape(TT * 8, 16).T, (8, 1))).astype(np.int16)

        batchloc = np.full(NSHP, -1.0, np.float32)
        batchloc[:NSH] = batch[c * NSH:(c + 1) * NSH]
        batchloc = np.ascontiguousarray(batchloc.reshape(NB, P).T)

        xT_own = np.zeros((IN, NSHP), np.float32)
        xT_own[:, :NSH] = xarr[c * NSH:(c + 1) * NSH].T

        in_maps.append(dict(
            xT=xT_own.astype(NPB), idx16=idx16, dstrow=dstrow,
            dstloc=dstloc.astype(NPB),
            iotap=np.arange(P, dtype=np.float32)[:, None],
            batchloc=batchloc, iota=iota, ident=ident,
            wt1=wt_aug[1], wt2=wt_aug[2], wt3=wt_aug[3],
            brep1=np.tile(W["b1"], (P, 1)).astype(NPB),
            brep2=np.tile(W["b2"], (P, 1)).astype(NPB),
            brep3=np.tile(W["b3"], (P, 1)).astype(NPB),
            wlt=np.ascontiguousarray(W["Wl"].T).astype(NPB),
            blrep=np.tile(W["bl"], (G, 1)).astype(np.float32),
            invcnt=invcnt, gcnt=gcnt_arr,
        ))
    return TBS, in_maps


def kernel(**inputs):
    x = inputs.pop("x")
    edge_index = inputs.pop("edge_index")
    batch = inputs.pop("batch")
    TBS, in_maps = preprocess(x, edge_index, batch, inputs)
    nc = build_program(TBS)
    res = run_bass_kernel_spmd(nc, in_maps, list(range(NCORES)))
    return np.asarray(res.results[0]["out"], np.float32)

